# revision 48
# baseline (speedup 1.0000x reference)
"""BiLSTM-CRF Trainium2 kernel (Bass/Tile), data-parallel over batch on 8
NeuronCores. Self-contained: host prep + device emission + SPMD runner.

Pipeline per core (16 sequences, T=512):
  embedding gather (indirect DMA, fp16) -> PE transpose -> Wx matmuls (fp16),
  software-pipelined with the serial BiLSTM recurrence -> emission scores
  (overlapped into the recurrence as hb becomes available) -> blocked Viterbi
  forward scan + blocked backtrace (max-plus / one-hot map composition in 32
  chunks of 16 steps, vectorized across 128 partitions).

Recurrence slot (both directions merged, latency-optimized):
  - all-tanh cell: host prep pre-halves i/f/o pre-activations so
    sigma(x) = (tanh(x/2)+1)/2; ONE scalar-engine Tanh covers all 4 gates.
    Hidden state is tracked doubled (H = 2h, Whh/W_lab pre-halved) and the
    cell state doubled (C = 2c), which makes the whole cell update 3 fused
    DVE scalar_tensor_tensor ops + one Tanh(scale=0.5).
  - wx is prefilled into the psum bank one slot ahead by ACT (fwd half)
    and DVE (bwd half); the PE matmul group is pure start=False on top
    (a 2nd start=True inside a group drops the first prefill on HW).
  - bwd pack-padding masking is a K=1 matmul adding -15 to i/f/o
    pre-activations at t >= len (tanh saturates to -1, state freezes at 0),
    replacing 12.6MB of per-core mask DMA traffic with a 64KB table.
Chain per step: 8 whh matmuls (~360ns) -> Tanh(gates) -> stt m12 ->
stt C -> Tanh(C/2) -> stt H -> next matmuls; ~2.09us/step, latency-bound.
"""
import sys
import types
import numpy as np

import concourse.bass as bass
import concourse.mybir as mybir
from concourse import tile
from concourse.vector_clock import ScopedClock
import bass_rust
from contextlib import ExitStack

F16 = mybir.dt.float16
F32 = mybir.dt.float32
I32 = mybir.dt.int32
AF = mybir.ActivationFunctionType
AX = mybir.AxisListType.X
OP = mybir.AluOpType

B_FULL, T, V, D = 128, 512, 8000, 256
NB = 16          # sequences per core
NCORES = 8


# ---------------------------------------------------------------------------
# Harness workarounds: walrus in this environment accepts only ONE sync-wait
# per instruction; split extras onto NoOps (BIR json pass) and chunk the Tile
# exit drain. Also register the NTFF profile hook shim so BASS_TRACE=1 works.
# ---------------------------------------------------------------------------
import json as _json

_SW_CTR = [0]


def _split_sync_waits(bir_json: bytes) -> bytes:
    d = _json.loads(bir_json)
    changed = False
    for fn in d.get("functions", []):
        for blk in fn.get("blocks", []):
            new_insts = []
            for inst in blk.get("instructions", []):
                si = inst.get("sync_info")
                waits = (si or {}).get("on_wait") or []
                if len(waits) > 1:
                    changed = True
                    for w in waits[:-1]:
                        _SW_CTR[0] += 1
                        nop = {
                            "engine": inst["engine"],
                            "ins": [],
                            "outs": [],
                            "name": f"I-swsplit-{_SW_CTR[0]}",
                            "opcode": "NoOp",
                            "sync_info": {"on_update": [], "on_wait": [w]},
                        }
                        if "debug" in inst:
                            nop["debug"] = inst["debug"]
                        new_insts.append(nop)
                    si["on_wait"] = [waits[-1]]
                new_insts.append(inst)
            blk["instructions"] = new_insts
    return _json.dumps(d).encode() if changed else bir_json


def _patched_drain_and_barrier(self, tick_clock, wait_clock):
    drain_inst = self.nc.sync.drain()
    wait_clock.add_sem_waits(
        drain_inst.ins, ScopedClock({None: tick_clock.global_clock})
    )
    si = drain_inst.ins.sync_info
    if si is not None and si.on_wait is not None and len(si.on_wait) > 1:
        waits = list(si.on_wait)
        drain_inst.ins.sync_info = bass_rust.SyncInfo(
            on_wait=waits[:1], on_update=list(si.on_update or [])
        )
        for i in range(1, len(waits)):
            nop = self.nc.sync.nop()
            nop.ins.sync_info = bass_rust.SyncInfo(on_wait=[waits[i]], on_update=[])
    self.nc.all_engine_barrier()
    assert self.sems is not None
    popped = self.nc._tile_sem_poison_stack.pop()
    assert popped is self._sem_poison
    self.nc.clear_and_free_semaphores(list(self.sems.allocated().values()))
    self.nc.all_engine_barrier()


_PATCHED = [False]


def _apply_patches():
    if _PATCHED[0]:
        return
    _PATCHED[0] = True
    tile.TileContext._drain_and_barrier = _patched_drain_and_barrier
    import concourse.bass_utils as _bu
    import concourse.bass2jax as _b2j

    _orig_compile = _bu.compile_bir_kernel

    def _wrapped(bir_json, tmpdir, neff_name="file.neff"):
        return _orig_compile(_split_sync_waits(bir_json), tmpdir, neff_name)

    _wrapped._swsplit_wrapped = True
    _bu.compile_bir_kernel = _wrapped
    _b2j.compile_bir_kernel = _wrapped

    if "antenv.axon_hooks" not in sys.modules:
        try:
            import trn_agent_boot.trn_boot as _tb
            _hook = _tb._ntff_profile_via_ctypes("/opt/axon/libaxon_pjrt.so")
        except Exception:
            _hook = None
        m = types.ModuleType("antenv.axon_hooks")
        m.get_axon_ntff_profile_hook = lambda: _hook
        m.set_axon_ntff_profile_hook = lambda h: None
        sys.modules["antenv.axon_hooks"] = m





def A(t, off, dims, p0=0):
    # t: pool tile AP [[rowsize, P], [1, rowsize]]. dims[0] is the partition
    # pair whose step is replaced by the tile's canonical per-partition row
    # size; off is the within-partition element offset.
    rs = t.ap[0][0]
    d = [list(x) for x in dims]
    d[0] = [rs, d[0][1]]
    return bass.AP(t.tensor, t.offset + p0 * rs + off, d)


def AD(handle, off, dims):
    return bass.AP(handle, off, [list(d) for d in dims])


def load_crf_consts(nc, dr, pool):
    """Allocate + DMA the lens-dependent / constant CRF tables early (during
    the recurrence) so the CRF phase never waits on them; vmask alone is
    512KB (~20us on one DMA queue)."""
    c = {}
    def bcast(name, p, w, dt=F32):
        t = pool.tile([p, w], dt, name="c_" + name)
        nc.sync.dma_start(t[:], dr[name][None, :].to_broadcast((p, w)))
        c[name] = t
    bcast("transb16", 128, 16)
    bcast("impflat", 128, 16)
    bcast("fromBp4", 128, 4)
    bcast("toEOS4", 16, 4)
    bcast("c3lab4", 16, 4)
    t = pool.tile([16, 4], F32, name="c_wiota4_16")
    nc.sync.dma_start(t[:], dr["wiota4"][None, :].to_broadcast((16, 4)))
    c["wiota4_16"] = t
    bcast("c3p16", 128, 16)
    bcast("i4flat", 128, 16)
    t2 = pool.tile([128, 4], F32, name="c_wiota4_128")
    nc.sync.dma_start(t2[:], dr["wiota4"][None, :].to_broadcast((128, 4)))
    c["wiota4_128"] = t2
    for name, sh, dt in (("vmask128", [128, 1024], I32),
                         ("meq128", [128, 64], I32),
                         ("mlt128", [128, 64], I32),
                         ("outmask128", [128, 64], F32)):
        t = pool.tile(sh, dt, name="c_" + name)
        nc.sync.dma_start(t[:], dr[name][:])
        c[name] = t
    # V0 working tiles, allocated early so the scT loads + T-matrix builds
    # for score groups 1..6 can run during the recurrence
    c["scT"] = pool.tile([128, 256], F32, name="c_scT")
    c["Traw"] = pool.tile([128, 1024], F32, name="c_Traw")
    c["T128"] = pool.tile([128, 1024], F32, name="c_T128")
    return c


def emit_v0_load(nc, dr, cs, g):
    """DMA score group g from DRAM into its scT slice (any partition start)."""
    nc.sync.dma_start(
        A(cs["scT"], 0, [[1, 16], [4, 64], [1, 4]], p0=g * 16),
        AD(dr["scores"], g * 4096, [[4, 16], [64, 64], [1, 4]]),
    )


def emit_v0_build(nc, cs, p0):
    """Build the T-matrix slice for a 32-partition group pair. Compute
    engines require quadrant-aligned partition starts (0/32/64/96)."""
    v = nc.vector
    scT, Traw, T128 = cs["scT"], cs["Traw"], cs["T128"]
    v.tensor_add(
        out=A(Traw, 0, [[1, 32], [256, 4], [16, 16], [4, 4], [1, 4]], p0=p0),
        in0=A(scT, 0, [[1, 32], [64, 4], [4, 16], [0, 4], [1, 4]], p0=p0),
        in1=A(cs["transb16"], 0, [[1, 32], [0, 4], [0, 16], [4, 4], [1, 4]], p0=p0),
    )
    v.select(
        out=A(T128, 0, [[1, 32], [256, 4], [16, 16], [4, 4], [1, 4]], p0=p0),
        mask=A(cs["vmask128"], 0, [[1, 32], [256, 4], [16, 16], [4, 4], [1, 4]], p0=p0),
        on_true=A(Traw, 0, [[1, 32], [256, 4], [16, 16], [4, 4], [1, 4]], p0=p0),
        on_false=A(cs["impflat"], 0, [[1, 32], [0, 4], [0, 16], [4, 4], [1, 4]], p0=p0),
    )


def emit_crf(nc, tc, dr, pool, cs):
    """dr: dict of DRAM handles. pool: sbuf tile pool to allocate from."""
    v = nc.vector

    # ---- V0: middle pairs were built during the recurrence; finish the
    # edge groups 0 and 7 (ready only at the very end) and their pairs --
    scT, Traw, T128 = cs["scT"], cs["Traw"], cs["T128"]
    fromBp_sb = cs["fromBp4"]
    for g in (0, 7):
        emit_v0_load(nc, dr, cs, g)
    emit_v0_build(nc, cs, 0)
    emit_v0_build(nc, cs, 96)
    # step 0 (partitions 0:16, chpos=0, s=0): T = e0 + fromBp (rows equal)
    v.tensor_add(
        out=A(T128, 0, [[1, 16], [4, 4], [1, 4]]),
        in0=A(scT, 0, [[1, 16], [0, 4], [1, 4]]),
        in1=A(fromBp_sb, 0, [[1, 16], [0, 4], [1, 4]]),
    )

    # ---- V1: chunk max-plus products -----------------------------------
    Ma = pool.tile([128, 64], F32)   # (chpos, i, k/j)
    Mb = pool.tile([128, 64], F32)
    tmp256 = pool.tile([128, 1024], F32)
    v.tensor_copy(
        A(Ma, 0, [[1, 128], [16, 4], [4, 4], [1, 4]]),
        A(T128, 0, [[1, 128], [256, 4], [4, 4], [1, 4]]),
    )
    cur, nxt = Ma, Mb
    for s in range(1, 16):
        v.tensor_add(
            out=A(tmp256, 0, [[1, 128], [64, 4], [16, 4], [4, 4], [1, 4]]),
            in0=A(cur, 0, [[1, 128], [16, 4], [4, 4], [1, 4], [0, 4]]),
            in1=A(T128, s * 16, [[1, 128], [256, 4], [0, 4], [4, 4], [1, 4]]),
        )
        v.tensor_reduce(
            out=A(nxt, 0, [[1, 128], [16, 4], [4, 4], [1, 4]]),
            in_=A(tmp256, 0, [[1, 128], [64, 4], [16, 4], [1, 4], [4, 4]]),
            axis=AX, op=OP.max,
        )
        cur, nxt = nxt, cur
    nc.sync.dma_start(
        AD(dr["mdram"], 0, [[64, 128], [1, 64]]),
        A(cur, 0, [[1, 128], [1, 64]]),
    )

    # ---- V2: serial chunk scan (16 partitions) -------------------------
    M16 = pool.tile([16, 512], F32)
    for g in range(8):
        nc.sync.dma_start(
            A(M16, g * 64, [[1, 16], [1, 64]]),
            AD(dr["mdram"], g * 1024, [[64, 16], [1, 64]]),
        )
    Ball = pool.tile([16, 132], F32)
    v.memset(Ball[:], 0.0)
    t16 = pool.tile([16, 16], F32)
    for c in range(32):
        v.tensor_add(
            out=A(t16, 0, [[1, 16], [4, 4], [1, 4]]),
            in0=A(Ball, c * 4, [[1, 16], [1, 4], [0, 4]]),
            in1=A(M16, c * 16, [[1, 16], [4, 4], [1, 4]]),
        )
        v.tensor_reduce(
            out=A(Ball, (c + 1) * 4, [[1, 16], [1, 4]]),
            in_=A(t16, 0, [[1, 16], [1, 4], [4, 4]]),
            axis=AX, op=OP.max,
        )
    # last label one-hot
    toEOS_sb = cs["toEOS4"]
    c3lab_sb = cs["c3lab4"]
    wiota16 = cs["wiota4_16"]
    fin = pool.tile([16, 4], F32)
    v.tensor_add(out=fin[:], in0=A(Ball, 128, [[1, 16], [1, 4]]), in1=toEOS_sb[:])
    lmax = pool.tile([16, 1], F32)
    v.tensor_reduce(out=lmax[:], in_=fin[:], axis=AX, op=OP.max)
    loh = pool.tile([16, 4], F32)
    v.tensor_tensor(out=loh[:], in0=fin[:],
                    in1=A(lmax, 0, [[1, 16], [0, 4]]), op=OP.is_equal)
    lohm = pool.tile([16, 4], F32)
    v.tensor_mul(out=lohm[:], in0=loh[:], in1=c3lab_sb[:])
    lenc = pool.tile([16, 1], F32)
    v.tensor_reduce(out=lenc[:], in_=lohm[:], axis=AX, op=OP.max)
    llval = pool.tile([16, 1], F32)
    v.tensor_scalar(out=llval[:], in0=lenc[:], scalar1=-1.0, scalar2=3.0,
                    op0=OP.mult, op1=OP.add)
    lloh = pool.tile([16, 4], F32)
    v.tensor_tensor(out=lloh[:], in0=wiota16[:],
                    in1=A(llval, 0, [[1, 16], [0, 4]]), op=OP.is_equal)
    nc.sync.dma_start(AD(dr["lldram"], 0, [[4, 16], [1, 4]]), lloh[:])
    nc.sync.dma_start(AD(dr["edram"], 0, [[132, 16], [1, 132]]), Ball[:])

    # ---- V3: replay -> backtrace tables --------------------------------
    c3p_sb = cs["c3p16"]
    bestA = pool.tile([128, 16], F32)
    bestB = pool.tile([128, 16], F32)
    nc.sync.dma_start(
        bestA[:], AD(dr["edram"], 0, [[16, 8], [132, 16], [4, 4], [1, 4]])
    )
    BT = pool.tile([128, 256], F32)     # (chpos, s, c)
    smat = pool.tile([128, 64], F32)
    oh64 = pool.tile([128, 64], F32)
    enc128 = pool.tile([128, 16], F32)
    bcur, bnxt = bestA, bestB
    for s in range(16):
        v.tensor_add(
            out=A(smat, 0, [[1, 128], [16, 4], [4, 4], [1, 4]]),
            in0=A(bcur, 0, [[1, 128], [4, 4], [1, 4], [0, 4]]),
            in1=A(T128, s * 16, [[1, 128], [256, 4], [4, 4], [1, 4]]),
        )
        v.tensor_reduce(
            out=A(bnxt, 0, [[1, 128], [4, 4], [1, 4]]),
            in_=A(smat, 0, [[1, 128], [16, 4], [1, 4], [4, 4]]),
            axis=AX, op=OP.max,
        )
        v.tensor_tensor(
            out=A(oh64, 0, [[1, 128], [16, 4], [4, 4], [1, 4]]),
            in0=A(smat, 0, [[1, 128], [16, 4], [4, 4], [1, 4]]),
            in1=A(bnxt, 0, [[1, 128], [4, 4], [0, 4], [1, 4]]),
            op=OP.is_equal,
        )
        v.tensor_mul(
            out=A(oh64, 0, [[1, 128], [16, 4], [4, 4], [1, 4]]),
            in0=A(oh64, 0, [[1, 128], [16, 4], [4, 4], [1, 4]]),
            in1=A(c3p_sb, 0, [[1, 128], [0, 4], [4, 4], [1, 4]]),
        )
        v.tensor_reduce(
            out=A(enc128, 0, [[1, 128], [4, 4], [1, 4]]),
            in_=A(oh64, 0, [[1, 128], [16, 4], [1, 4], [4, 4]]),
            axis=AX, op=OP.max,
        )
        v.tensor_scalar(
            out=A(BT, s * 4, [[1, 128], [64, 4], [1, 4]]),
            in0=A(enc128, 0, [[1, 128], [4, 4], [1, 4]]),
            scalar1=-1.0, scalar2=3.0, op0=OP.mult, op1=OP.add,
        )
        bcur, bnxt = bnxt, bcur
    nc.sync.dma_start(
        AD(dr["btdram"], 0, [[256, 128], [1, 256]]),
        A(BT, 0, [[1, 128], [1, 256]]),
    )

    # ---- VA: backtrace map tables + chunk compositions -----------------
    BTS = pool.tile([128, 256], F32)
    # top group's last slot is never used; zero-fill before partial overwrite
    v.memset(A(BTS, 252, [[1, 128], [1, 4]]), 0.0)
    nc.sync.dma_start(
        A(BTS, 0, [[1, 128], [1, 252]]),
        AD(dr["btdram"], 4, [[256, 128], [1, 252]]),
    )
    # last slot of each partition: first bt entry of the next chunk group
    nc.sync.dma_start(
        A(BTS, 252, [[1, 112], [1, 4]]),
        AD(dr["btdram"], 16 * 256, [[256, 112], [1, 4]]),
    )
    meq_sb = cs["meq128"]
    mlt_sb = cs["mlt128"]
    lloh128 = pool.tile([128, 4], F32)
    nc.sync.dma_start(lloh128[:], AD(dr["lldram"], 0, [[0, 8], [4, 16], [1, 4]]))
    i4_sb = cs["i4flat"]
    wiota128 = cs["wiota4_128"]

    Fall = pool.tile([128, 1024], F32)  # (chpos, s, u, w)
    tmpA = pool.tile([128, 64], F32)
    for s in range(16):
        # oh(u,w) = bt_{t+1}[u] == w
        v.tensor_tensor(
            out=A(tmpA, 0, [[1, 128], [16, 4], [4, 4], [1, 4]]),
            in0=A(BTS, s * 4, [[1, 128], [64, 4], [1, 4], [0, 4]]),
            in1=A(wiota128, 0, [[1, 128], [0, 4], [0, 4], [1, 4]]),
            op=OP.is_equal,
        )
        # tmp2 = meq ? lloh : I4  ; F = mlt ? oh : tmp2  (write into Fall)
        v.select(
            out=A(Fall, s * 16, [[1, 128], [256, 4], [4, 4], [1, 4]]),
            mask=A(meq_sb, s, [[1, 128], [16, 4], [0, 4], [0, 4]]),
            on_true=A(lloh128, 0, [[1, 128], [0, 4], [0, 4], [1, 4]]),
            on_false=A(i4_sb, 0, [[1, 128], [0, 4], [4, 4], [1, 4]]),
        )
        v.select(
            out=A(Fall, s * 16, [[1, 128], [256, 4], [4, 4], [1, 4]]),
            mask=A(mlt_sb, s, [[1, 128], [16, 4], [0, 4], [0, 4]]),
            on_true=A(tmpA, 0, [[1, 128], [16, 4], [4, 4], [1, 4]]),
            on_false=A(Fall, s * 16, [[1, 128], [256, 4], [4, 4], [1, 4]]),
        )
    Ga = pool.tile([128, 64], F32)
    Gb = pool.tile([128, 64], F32)
    v.tensor_copy(
        A(Ga, 0, [[1, 128], [16, 4], [4, 4], [1, 4]]),
        A(Fall, 15 * 16, [[1, 128], [256, 4], [4, 4], [1, 4]]),
    )
    gcur, gnxt = Ga, Gb
    for s in range(14, -1, -1):
        v.tensor_mul(
            out=A(tmp256, 0, [[1, 128], [64, 4], [16, 4], [4, 4], [1, 4]]),
            in0=A(gcur, 0, [[1, 128], [16, 4], [4, 4], [1, 4], [0, 4]]),
            in1=A(Fall, s * 16, [[1, 128], [256, 4], [0, 4], [4, 4], [1, 4]]),
        )
        v.tensor_reduce(
            out=A(gnxt, 0, [[1, 128], [16, 4], [4, 4], [1, 4]]),
            in_=A(tmp256, 0, [[1, 128], [64, 4], [16, 4], [1, 4], [4, 4]]),
            axis=AX, op=OP.max,
        )
        gcur, gnxt = gnxt, gcur
    nc.sync.dma_start(
        AD(dr["gdram"], 0, [[64, 128], [1, 64]]),
        A(gcur, 0, [[1, 128], [1, 64]]),
    )

    # ---- VB: serial reverse chunk scan (16 partitions) -----------------
    Gall16 = pool.tile([16, 512], F32)
    for g in range(8):
        nc.sync.dma_start(
            A(Gall16, g * 64, [[1, 16], [1, 64]]),
            AD(dr["gdram"], g * 1024, [[64, 16], [1, 64]]),
        )
    EB = pool.tile([16, 132], F32)
    nc.sync.dma_start(
        A(EB, 128, [[1, 16], [1, 4]]), dr["e0oh4"][None, :].to_broadcast((16, 4))
    )
    tb16 = pool.tile([16, 16], F32)
    for c in range(31, -1, -1):
        v.tensor_mul(
            out=tb16[:],
            in0=A(EB, (c + 1) * 4, [[1, 16], [1, 4], [0, 4]]),
            in1=A(Gall16, c * 16, [[1, 16], [4, 4], [1, 4]]),
        )
        v.tensor_reduce(
            out=A(EB, c * 4, [[1, 16], [1, 4]]),
            in_=A(tb16, 0, [[1, 16], [1, 4], [4, 4]]),
            axis=AX, op=OP.max,
        )
    nc.sync.dma_start(AD(dr["ebdram"], 0, [[132, 16], [1, 132]]), EB[:])

    # ---- VC: labels -----------------------------------------------------
    cohE = pool.tile([128, 16], F32)
    nc.sync.dma_start(
        cohE[:], AD(dr["ebdram"], 4, [[16, 8], [132, 16], [4, 4], [1, 4]])
    )
    LABOH = pool.tile([128, 256], F32)  # (chpos, s, w)
    tmpc = pool.tile([128, 64], F32)
    for s in range(15, -1, -1):
        if s == 15:
            in0 = A(cohE, 0, [[1, 128], [4, 4], [1, 4], [0, 4]])
        else:
            in0 = A(LABOH, (s + 1) * 4, [[1, 128], [64, 4], [1, 4], [0, 4]])
        v.tensor_mul(
            out=A(tmpc, 0, [[1, 128], [16, 4], [4, 4], [1, 4]]),
            in0=in0,
            in1=A(Fall, s * 16, [[1, 128], [256, 4], [4, 4], [1, 4]]),
        )
        v.tensor_reduce(
            out=A(LABOH, s * 4, [[1, 128], [64, 4], [1, 4]]),
            in_=A(tmpc, 0, [[1, 128], [16, 4], [1, 4], [4, 4]]),
            axis=AX, op=OP.max,
        )
    omask_sb = cs["outmask128"]
    labv = pool.tile([128, 64], F32)
    tmpl = pool.tile([128, 256], F32)
    v.tensor_mul(
        out=A(tmpl, 0, [[1, 128], [64, 4], [4, 16], [1, 4]]),
        in0=A(LABOH, 0, [[1, 128], [64, 4], [4, 16], [1, 4]]),
        in1=A(wiota128, 0, [[1, 128], [0, 4], [0, 16], [1, 4]]),
    )
    v.tensor_reduce(
        out=A(labv, 0, [[1, 128], [16, 4], [1, 16]]),
        in_=A(tmpl, 0, [[1, 128], [64, 4], [4, 16], [1, 4]]),
        axis=AX, op=OP.add,
    )
    v.tensor_mul(out=labv[:], in0=labv[:], in1=omask_sb[:])
    labi = pool.tile([128, 64], I32)
    v.tensor_copy(labi[:], labv[:])
    for cp in range(4):
        nc.sync.dma_start(
            AD(dr["labels"], 16 * cp, [[64, 8], [512, 16], [1, 16]]),
            A(labi, cp * 16, [[1, 128], [1, 16]]),
        )


def host_crf_consts(lens, trans, fromB, toEOS, b_lab):
    """All host-side constant arrays, keyed to match dram handle names."""
    import numpy as np
    T, B, L = 512, 16, 4
    NEG = -1e9
    chgrp = np.arange(8)
    out = {}
    out["transb16"] = (trans + b_lab[None, :]).astype(np.float32).reshape(16)
    imp = np.full((L, L), NEG, np.float32)
    np.fill_diagonal(imp, 0.0)
    out["impflat"] = imp.reshape(16)
    out["fromBp4"] = (fromB + b_lab).astype(np.float32)
    out["toEOS4"] = toEOS.astype(np.float32)
    out["c3lab4"] = (3.0 - np.arange(4)).astype(np.float32)
    out["wiota4"] = np.arange(4).astype(np.float32)
    out["c3p16"] = np.repeat(3.0 - np.arange(4), 4).astype(np.float32)
    out["i4flat"] = np.eye(4, dtype=np.float32).reshape(16)
    out["e0oh4"] = np.array([1, 0, 0, 0], np.float32)
    # t value at (P, chpos, s):  P = chgrp*16 + b ; t = 16*(4*chgrp+chpos)+s
    P_chgrp = np.arange(128) // 16
    P_b = np.arange(128) % 16
    chpos = np.arange(4)
    s = np.arange(16)
    tt = 16 * (4 * P_chgrp[:, None, None] + chpos[None, :, None]) + s[None, None, :]
    lb = lens[P_b][:, None, None]
    vm = (tt < lb)
    out["vmask128"] = np.repeat(
        vm.reshape(128, 64)[:, :, None], 16, axis=2
    ).reshape(128, 1024).astype(np.int32)
    out["meq128"] = (tt == lb - 1).reshape(128, 64).astype(np.int32)
    out["mlt128"] = (tt < lb - 1).reshape(128, 64).astype(np.int32)
    out["outmask128"] = (tt < lb).reshape(128, 64).astype(np.float32)
    return out


CRF_DRAM_SPECS = [
    ("transb16", [16], F32), ("impflat", [16], F32), ("fromBp4", [4], F32),
    ("toEOS4", [4], F32), ("c3lab4", [4], F32), ("wiota4", [4], F32),
    ("c3p16", [16], F32), ("i4flat", [16], F32), ("e0oh4", [4], F32),
    ("vmask128", [128, 1024], I32), ("meq128", [128, 64], I32),
    ("mlt128", [128, 64], I32), ("outmask128", [128, 64], F32),
]
CRF_SCRATCH_SPECS = [
    ("mdram", [8192], F32), ("edram", [2112], F32), ("btdram", [32832], F32),
    ("gdram", [8192], F32), ("lldram", [64], F32), ("ebdram", [2112], F32),
]






class LstmEmitter:
    def __init__(self, nc, tc, dr, T, pools):
        self.nc, self.tc, self.dr, self.T = nc, tc, dr, T
        self.NBLK = T // 64
        p = pools
        hist_all = p["hist"].tile([128, 2 * (T + 1) * 16], F16, name="hist_all")
        self.hist_all = hist_all
        self.hb_off = (T + 1) * 16
        self.hist = {
            "f": hist_all[:, 0:(T + 1) * 16],
            "b": hist_all[:, (T + 1) * 16:2 * (T + 1) * 16],
        }
        nc.vector.memset(self.hist["f"][:, 0:16], 0.0)
        nc.vector.memset(self.hist["b"][:, T * 16:(T + 1) * 16], 0.0)
        # tall layout (dir-major, doubled cell state C=2c interleaved so
        # every DVE operand is a <=3-dim AP):
        #   [i_f f_f o_f g_f](0:64) [C_f](64:80) [i_b f_b o_b g_b](80:144)
        #   [C_b](144:160)
        self.tall = p["state"].tile([128, 160], F16, name="tall")
        nc.vector.memset(A(self.tall, 64, [[1, 128], [80, 2], [1, 16]]), 0.0)
        self.g_ps_by_slot = {}
        # weights
        self.whhT = {}
        self.wihT = {}
        self.biasT = {}
        self.wlabT = {}
        self.pools = p
        self.wx = {}   # (dir, blk) -> tile [128, 4096] fp16, chunk-major

    # Weight/const DMAs are emitted AFTER the prologue's gather DMAs so the
    # tiny token-index DMAs are not stuck behind ~40us of weight traffic.
    def load_ident(self):
        nc, dr, p = self.nc, self.dr, self.pools
        self.ident = p["wts"].tile([128, 128], F16, name="ident")
        nc.scalar.dma_start(self.ident[:], dr["ident"][:])

    def load_weights(self):
        nc, dr, p = self.nc, self.dr, self.pools
        for d in ("f", "b"):
            w = p["wts"].tile([128, 512], F16, name=f"whh_{d}")
            nc.sync.dma_start(w[:], dr[f"whhT_{d}"][:])
            self.whhT[d] = w
            hs = []
            for h in range(2):
                wh = p["wts"].tile([128, 512], F16, name=f"wih_{d}{h}")
                nc.sync.dma_start(wh[:], dr[f"wihT_{d}{h}"][:])
                hs.append(wh)
            self.wihT[d] = hs
            bt = p["wts"].tile([128, 4], F32, name=f"bias_{d}")
            nc.sync.dma_start(bt[:], dr[f"biasT_{d}"][:])
            self.biasT[d] = bt
            wl = p["wts"].tile([128, 4], F16, name=f"wlab_{d}")
            nc.sync.dma_start(wl[:], dr[f"wlabT_{d}"][:])
            self.wlabT[d] = wl
        self.masktab = p["wts"].tile([1, 512 * 64], F16, name="masktab")
        nc.sync.dma_start(self.masktab[:], dr["masktab"][None, :])
        self.ones1 = p["wts"].tile([1, 128], F16, name="ones1")
        nc.vector.memset(self.ones1[:], 1.0)

    # ---- production of one dir-block's wx ------------------------------
    def production_items(self, d, blk):
        """Returns a list of closures emitting the gather/transpose/matmul/
        bias work that materializes wx[d][blk]."""
        nc, dr, p = self.nc, self.dr, self.pools
        items = []
        xg = []      # gather tiles [128, 256] fp16
        xt = None    # XT tile [128, 2048] fp16 (halves at 0 / 1024)
        wxt = None   # wx out tile [128, 4096] fp16
        state = {}

        def alloc():
            state["xt"] = p["xt"].tile([128, 2048], F16, name="xt")
            state["wx"] = p[f"wx_{d}"].tile([128, 4096], F16, name=f"wx_{d}")
            self.wx[(d, blk)] = state["wx"]

        idxs = {}

        def gather_idx(i):
            def go():
                idx = p["idx"].tile([128, 1], I32)
                nc.sync.dma_start(
                    idx[:],
                    bass.AP(dr["tokens"], blk * 1024 + i * 128, [[1, 128], [1, 1]]),
                )
                idxs[i] = idx
            return go

        def gather(i):
            def go():
                t = p["xg"].tile([128, 256], F16)
                nc.gpsimd.indirect_dma_start(
                    out=t[:], out_offset=None, in_=dr["emb16"][:],
                    in_offset=bass.IndirectOffsetOnAxis(ap=idxs.pop(i)[:, :1], axis=0),
                )
                xg.append(t)
            return go

        def transp(i, h):
            def go():
                ps = p["tp_ps"].tile([128, 128], F16)
                nc.tensor.transpose(
                    out=ps[:], in_=xg[i][:, h * 128:(h + 1) * 128],
                    identity=self.ident[:],
                )
                # psum->sbuf move on DVE (gpsimd has no PSUM port) to keep
                # the scalar engine free for the recurrence chain
                nc.vector.tensor_copy(
                    state["xt"][:, h * 1024 + i * 128: h * 1024 + (i + 1) * 128],
                    ps[:],
                )
            return go

        def mm(j, n):
            def go():
                ps = p["wx_ps"].tile([128, 512], F32)
                state[("ps", j, n)] = ps
                for h in range(2):
                    nc.tensor.matmul(
                        out=ps[:],
                        lhsT=self.wihT[d][h][:, j * 128:(j + 1) * 128],
                        rhs=state["xt"][:, h * 1024 + n * 512: h * 1024 + (n + 1) * 512],
                        start=(h == 0), stop=(h == 1),
                    )
            return go

        def bias(j, n):
            def go():
                nc.scalar.activation(
                    out=state["wx"][:, j * 1024 + n * 512: j * 1024 + (n + 1) * 512],
                    in_=state[("ps", j, n)][:],
                    func=AF.Identity, bias=self.biasT[d][:, j:j + 1],
                )
            return go

        items.append(alloc)
        for i in range(8):
            items.append(gather_idx(i))
        for i in range(8):
            items.append(gather(i))
        for i in range(8):
            for h in range(2):
                items.append(transp(i, h))
        for j in range(4):
            for n in range(2):
                items.append(mm(j, n))
                items.append(bias(j, n))
        return items

    # ---- one recurrence slot: fwd step t_f and bwd step t_b merged ----
    # All-tanh cell: host prep halves i/f/o pre-activations, so
    # sigma(x) = (tanh(x/2)+1)/2 and one Tanh covers all 4 gates. Hidden
    # state is tracked doubled (H = 2h, Whh/W_lab pre-halved) and cell
    # state doubled (C = 2c).
    # PSUM layout dir-major: cols 0:64 fwd gates [i f o g]x16, 64:128 bwd.
    # The wx prefill matmuls have no H dependence, so the in-order PE
    # executes them during the previous slot's ACT/DVE phase.
    def slot(self, t_f):
        nc, p, T = self.nc, self.pools, self.T
        t_b = T - 1 - t_f
        ha = self.hist_all
        hb0 = self.hb_off
        hprev = {"f": ha[:, t_f * 16:(t_f + 1) * 16],
                 "b": ha[:, hb0 + (t_b + 1) * 16:hb0 + (t_b + 2) * 16]}
        g_ps = self.g_ps_by_slot.pop(t_f)
        # The wx prefill was written into the psum bank by ACT/DVE one slot
        # ago, so the whole matmul group is start=False on top of it (a
        # matmul prefill is not hoistable: a second start=True drops the
        # first group's prefill, verified on hardware).
        # bwd pack-padding: add -15 to i/f/o pre-activations at t >= len
        # via a K=1 matmul from a tiny per-t table. No H dependence, so
        # the in-order PE runs it before the H-dependent whh run.
        nc.tensor.matmul(
            out=g_ps[:, 64:128],
            lhsT=self.ones1[:],
            rhs=A(self.masktab, t_b * 64, [[1, 1], [1, 64]]),
            start=False, stop=False,
        )
        for d, off in (("f", 0), ("b", 64)):
            for j in range(4):
                nc.tensor.matmul(
                    out=g_ps[:, off + j * 16:off + (j + 1) * 16],
                    lhsT=self.whhT[d][:, j * 128:(j + 1) * 128],
                    rhs=hprev[d], start=False,
                    stop=(d == "b" and j == 3),
                )
        tall = self.tall
        nc.scalar.activation(
            out=A(tall, 0, [[1, 128], [80, 2], [1, 64]]),
            in_=g_ps[:], func=AF.Tanh,
        )
        # m12 = (in0+1)*in1 with in0 = [ti|tf], in1 = [tg|C] per dir
        # -> m12 cols: [m1_f](0:16) [m2_f](16:32) [m1_b](32:48) [m2_b](48:64)
        m12 = p["m12"].tile([128, 64], F16, name="m12")
        nc.vector.scalar_tensor_tensor(
            out=m12[:],
            in0=A(tall, 0, [[1, 128], [80, 2], [1, 32]]),
            scalar=1.0,
            in1=A(tall, 48, [[1, 128], [80, 2], [1, 32]]),
            op0=OP.add, op1=OP.mult,
        )
        # C_new = 2c_new = (ti+1)*tg + (tf+1)*C/2
        nc.vector.scalar_tensor_tensor(
            out=A(tall, 64, [[1, 128], [80, 2], [1, 16]]),
            in0=A(m12, 16, [[1, 128], [32, 2], [1, 16]]),
            scalar=0.5,
            in1=A(m12, 0, [[1, 128], [32, 2], [1, 16]]),
            op0=OP.mult, op1=OP.add,
        )
        tc_ = p["tc"].tile([128, 32], F16, name="tc")
        nc.scalar.activation(
            out=tc_[:], in_=A(tall, 64, [[1, 128], [80, 2], [1, 16]]),
            func=AF.Tanh, scale=0.5,
        )
        # H = (to+1)*tanh(c) = 2h, stored to both dirs' history in one op
        dlt = hb0 + t_b * 16 - (t_f + 1) * 16
        nc.vector.scalar_tensor_tensor(
            out=A(ha, (t_f + 1) * 16, [[1, 128], [dlt, 2], [1, 16]]),
            in0=A(tall, 32, [[1, 128], [80, 2], [1, 16]]),
            scalar=1.0,
            in1=tc_[:],
            op0=OP.add, op1=OP.mult,
        )
        if t_f + 1 < T:
            self.prefill(t_f + 1)

    # wx psum prefill for slot t: fwd half written by the scalar engine,
    # bwd half by DVE. Both run in their engines' idle window (no H
    # dependence) and complete before the PE's start=False matmul group
    # accumulates on top.
    def prefill(self, t_f):
        nc, p, T = self.nc, self.pools, self.T
        t_b = T - 1 - t_f
        g_ps = p["gps"].tile([128, 128], F32, name="g_ps")
        self.g_ps_by_slot[t_f] = g_ps
        wxf = self.wx[("f", t_f // 64)]
        nc.scalar.activation(
            out=g_ps[:, 0:64],
            in_=A(wxf, (t_f % 64) * 16, [[1, 128], [1024, 4], [1, 16]]),
            func=AF.Copy,
        )
        wxb = self.wx[("b", t_b // 64)]
        nc.vector.tensor_copy(
            g_ps[:, 64:128],
            A(wxb, (t_b % 64) * 16, [[1, 128], [1024, 4], [1, 16]]),
        )

    # ---- full pipelined emission ---------------------------------------
    def emit_recurrence(self):
        T, NBLK = self.T, self.NBLK
        # prologue: produce fwd block 0 and bwd block NBLK-1, interleaved so
        # the gathers/transposes/matmuls of both blocks overlap
        pf = self.production_items("f", 0)
        pb = self.production_items("b", NBLK - 1)
        # items: [0]=alloc, [1:9]=gathers, [9:]=transposes/matmuls/bias.
        # All 16 gather DMAs go first (the serial gpsimd queue is the
        # prologue's critical path), then the weight loads, then the rest.
        self.load_ident()
        pf[0](); pb[0]()
        for i in range(1, 9):
            pf[i]()
        for i in range(1, 9):
            pb[i]()
        self.load_weights()
        self.crf_consts = load_crf_consts(self.nc, self.dr,
                                          self.pools["wts"])
        for i in range(9, max(len(pf), len(pb))):
            if i < len(pf):
                pf[i]()
            if i < len(pb):
                pb[i]()
        self.prefill(0)
        # score chunk n becomes computable after slot max(8n+7, T-1-8n)
        ready = {}
        for n in range(T * 16 // 128):
            ready.setdefault(max(8 * n + 7, T - 1 - 8 * n), []).append(n)
        for blk in range(NBLK):
            todo = []
            if blk + 1 < NBLK:
                todo += self.production_items("f", blk + 1)
                todo += self.production_items("b", NBLK - 2 - blk)
            k = 0
            for tin in range(64):
                t = blk * 64 + tin
                if tin == 63:
                    # the next block's first slot reads the next blocks' wx:
                    # emit every production item first
                    while k < len(todo):
                        todo[k]()
                        k += 1
                self.slot(t)
                for n in ready.get(t, []):
                    self.emit_score_chunk(n)
                if 456 <= t <= 476 and t % 4 == 0:
                    # score groups 1..6 are all on DRAM by ~slot 450: load
                    # them and build the middle T-matrix quadrants in the
                    # recurrence's idle windows
                    g = (t - 456) // 4 + 1
                    emit_v0_load(self.nc, self.dr, self.crf_consts, g)
                    if g == 3:
                        emit_v0_build(self.nc, self.crf_consts, 32)
                    if g == 5:
                        emit_v0_build(self.nc, self.crf_consts, 64)
                want = ((tin + 1) * len(todo)) // 64
                while k < want:
                    todo[k]()
                    k += 1

    def emit_score_chunk(self, n):
        # emission scores for t in [8n, 8n+8): ready once hf[8n+7] (slot
        # 8n+7) and hb[8n] (slot T-1-8n) both exist
        nc, p = self.nc, self.pools
        ps = p["sc_ps"].tile([128, 4], F32, name="sc_ps")
        nc.tensor.matmul(out=ps[:], lhsT=self.hist["f"][:, 16 + n * 128: 16 + (n + 1) * 128],
                         rhs=self.wlabT["f"][:], start=True, stop=False)
        nc.tensor.matmul(out=ps[:], lhsT=self.hist["b"][:, n * 128:(n + 1) * 128],
                         rhs=self.wlabT["b"][:], start=False, stop=True)
        sb = p["sc_sb"].tile([128, 4], F32, name="sc_sb")
        nc.vector.tensor_copy(sb[:], ps[:])
        nc.sync.dma_start(
            bass.AP(self.dr["scores"], n * 512, [[4, 128], [1, 4]]), sb[:]
        )


def build_masktab(lens, T):
    import numpy as np
    # per-(t, j, seq) additive bwd mask: -15 on i/f/o gates at t >= len[b]
    # (tanh saturates to -1 so the gates close and state self-freezes to 0)
    t = np.arange(T)
    pad = (t[:, None] >= lens[None, :])            # [T, 16]
    m = np.zeros((T, 4, 16), np.float16)
    m[:, :3, :] = np.where(pad, -15.0, 0.0)[:, None, :]
    return m.reshape(-1)


def lstm_dram_specs(T=512):
    return [
        ("emb16", [8000, 256], F16), ("tokens", [T * 16], I32),
        ("wihT_f0", [128, 512], F16), ("wihT_f1", [128, 512], F16),
        ("wihT_b0", [128, 512], F16), ("wihT_b1", [128, 512], F16),
        ("whhT_f", [128, 512], F16), ("whhT_b", [128, 512], F16),
        ("biasT_f", [128, 4], F32), ("biasT_b", [128, 4], F32),
        ("wlabT_f", [128, 4], F16), ("wlabT_b", [128, 4], F16),
        ("ident", [128, 128], F16),
        ("masktab", [T * 64], F16),
    ]


def make_pools(ctx_persist, ctx_trans, tc):
    p = {}
    p["hist"] = ctx_persist.enter_context(tc.tile_pool(name="hist", bufs=1))
    p["state"] = ctx_persist.enter_context(tc.tile_pool(name="state", bufs=1))
    p["wts"] = ctx_persist.enter_context(tc.tile_pool(name="wts", bufs=1))
    p["idx"] = ctx_trans.enter_context(tc.tile_pool(name="idx", bufs=32))
    p["xg"] = ctx_trans.enter_context(tc.tile_pool(name="xg", bufs=16))
    p["xt"] = ctx_trans.enter_context(tc.tile_pool(name="xt", bufs=2))
    p["wx_f"] = ctx_trans.enter_context(tc.tile_pool(name="wx_f", bufs=2))
    p["wx_b"] = ctx_trans.enter_context(tc.tile_pool(name="wx_b", bufs=2))
    p["tp_ps"] = ctx_trans.enter_context(tc.tile_pool(name="tp_ps", bufs=2, space="PSUM"))
    p["wx_ps"] = ctx_trans.enter_context(tc.tile_pool(name="wx_ps", bufs=2, space="PSUM"))
    p["gps"] = ctx_trans.enter_context(tc.tile_pool(name="gps", bufs=3, space="PSUM"))
    for nm in ("m12", "tc"):
        p[nm] = ctx_trans.enter_context(tc.tile_pool(name=nm, bufs=2))
    return p


def make_score_pools(ctx, tc):
    p = {}
    p["sc_ps"] = ctx.enter_context(tc.tile_pool(name="sc_ps", bufs=1, space="PSUM"))
    p["sc_sb"] = ctx.enter_context(tc.tile_pool(name="sc_sb", bufs=2))
    return p


# ---------------------------------------------------------------------------
# DRAM declarations + host prep + SPMD driver
# ---------------------------------------------------------------------------

def _build_program():
    nc = bass.Bass(trn_type="TRN2")
    dr = {}
    for name, shape, dt in lstm_dram_specs(T):
        dr[name] = nc.dram_tensor(name, shape, dt, kind="ExternalInput")
    for name, shape, dt in CRF_DRAM_SPECS:
        dr[name] = nc.dram_tensor(name, shape, dt, kind="ExternalInput")
    for name, shape, dt in CRF_SCRATCH_SPECS:
        dr[name] = nc.dram_tensor(name, shape, dt)
    import os as _os
    if _os.environ.get("BASSDBG_SCORES"):
        dr["scores"] = nc.dram_tensor("scores", [T * 16, 4], F32,
                                      kind="ExternalOutput")
    else:
        dr["scores"] = nc.dram_tensor("scores", [T * 16, 4], F32)
    dr["labels"] = nc.dram_tensor("labels", [NB, T], I32, kind="ExternalOutput")

    with tile.TileContext(nc) as tc:
        with ExitStack() as ctx:
            with ExitStack() as ctx_trans:
                pools = make_pools(ctx, ctx_trans, tc)
                pools.update(make_score_pools(ctx_trans, tc))
                em = LstmEmitter(nc, tc, dr, T, pools)
                em.emit_recurrence()
            with ExitStack() as ctx_crf:
                crf_pool = ctx_crf.enter_context(tc.tile_pool(name="crf", bufs=1))
                emit_crf(nc, tc, dr, crf_pool, em.crf_consts)
    return nc


_CACHE = {}
LAST_EXEC_NS = None


def kernel(**inputs):
    global LAST_EXEC_NS
    _apply_patches()
    from concourse.bass_utils import run_bass_kernel_spmd

    inp = {k: np.asarray(v) for k, v in inputs.items()}
    if "nc" not in _CACHE:
        _CACHE["nc"] = _build_program()
    nc = _CACHE["nc"]

    # shared (batch-independent) host arrays.
    # All-tanh trick: i/f/o rows are pre-halved so sigma(x)=(tanh(x/2)+1)/2
    # comes out of a single Tanh; the hidden state is tracked doubled
    # (H=2h), compensated by halving everything that consumes h.
    shared = {}
    shared["emb16"] = inp["emb"].astype(np.float16)
    perm = np.concatenate([np.arange(128), 128 + np.arange(128),
                           384 + np.arange(128), 256 + np.arange(128)])
    halfrow = np.ones((512, 1), np.float32)
    halfrow[:384] = 0.5          # i,f,o rows (post-perm order i,f,o,g)
    for d, sfx in (("f", "_f"), ("b", "_b")):
        wih = inp[f"W_ih{sfx}"][perm].astype(np.float32) * halfrow
        whh = inp[f"W_hh{sfx}"][perm].astype(np.float32) * halfrow * 0.5
        bias = ((inp[f"b_ih{sfx}"] + inp[f"b_hh{sfx}"])[perm]
                .astype(np.float32) * halfrow[:, 0])
        shared[f"wihT_{d}0"] = np.ascontiguousarray(wih.T[:128]).astype(np.float16)
        shared[f"wihT_{d}1"] = np.ascontiguousarray(wih.T[128:]).astype(np.float16)
        shared[f"whhT_{d}"] = np.ascontiguousarray(whh.T).astype(np.float16)
        shared[f"biasT_{d}"] = np.ascontiguousarray(
            bias.reshape(4, 128).T).astype(np.float32)
        wl = inp["W_lab"].astype(np.float32) * 0.5
        half = wl[:, :128] if d == "f" else wl[:, 128:]
        shared[f"wlabT_{d}"] = np.ascontiguousarray(half.T).astype(np.float16)
    shared["ident"] = np.eye(128, dtype=np.float16)

    trans = inp["transitions"].astype(np.float32)
    fromB = inp["from_BOS"].astype(np.float32)
    toEOS = inp["to_EOS"].astype(np.float32)
    b_lab = inp["b_lab"].astype(np.float32)

    pad_seq = inp["pad_seq"].astype(np.int64)
    lens_full = inp["lens"].astype(np.int64)

    in_maps = []
    for core in range(NCORES):
        b0 = core * NB
        seq = pad_seq[b0:b0 + NB]
        lens = lens_full[b0:b0 + NB]
        m = dict(shared)
        m["tokens"] = np.ascontiguousarray(seq.T).reshape(-1).astype(np.int32)
        m["masktab"] = build_masktab(lens, T)
        m.update(host_crf_consts(lens, trans, fromB, toEOS, b_lab))
        in_maps.append(m)

    res = run_bass_kernel_spmd(nc, in_maps, list(range(NCORES)))
    LAST_EXEC_NS = res.exec_time_ns
    out = np.concatenate([res.results[c]["labels"] for c in range(NCORES)], axis=0)
    return out.astype(np.int32)



# revision 49
# speedup vs baseline: 1.0121x; 1.0121x over previous
"""BiLSTM-CRF Trainium2 kernel (Bass/Tile), data-parallel over batch on 8
NeuronCores. Self-contained: host prep + device emission + SPMD runner.

Pipeline per core (16 sequences, T=512):
  embedding gather (indirect DMA, fp16) -> PE transpose -> Wx matmuls (fp16),
  software-pipelined with the serial BiLSTM recurrence -> emission scores
  (overlapped into the recurrence as hb becomes available) -> blocked Viterbi
  forward scan + blocked backtrace (max-plus / one-hot map composition in 32
  chunks of 16 steps, vectorized across 128 partitions).

Recurrence slot (both directions merged, latency-optimized):
  - all-tanh cell: host prep pre-halves i/f/o pre-activations so
    sigma(x) = (tanh(x/2)+1)/2; ONE scalar-engine Tanh covers all 4 gates.
    Hidden state is tracked doubled (H = 2h, Whh/W_lab pre-halved) and the
    cell state doubled (C = 2c), which makes the whole cell update 3 fused
    DVE scalar_tensor_tensor ops + one Tanh(scale=0.5).
  - wx is prefilled into the psum bank one slot ahead by ACT (fwd half)
    and DVE (bwd half); the PE matmul group is pure start=False on top
    (a 2nd start=True inside a group drops the first prefill on HW).
  - bwd pack-padding masking is a K=1 matmul adding -15 to i/f/o
    pre-activations at t >= len (tanh saturates to -1, state freezes at 0),
    replacing 12.6MB of per-core mask DMA traffic with a 64KB table.
Chain per step: 8 whh matmuls (~360ns) -> Tanh(gates) -> stt m12 ->
stt C -> Tanh(C/2) -> stt H -> next matmuls; ~2.09us/step, latency-bound.
"""
import sys
import types
import numpy as np

import concourse.bass as bass
import concourse.mybir as mybir
from concourse import tile
from concourse.vector_clock import ScopedClock
import bass_rust
from contextlib import ExitStack

F16 = mybir.dt.float16
F32 = mybir.dt.float32
I32 = mybir.dt.int32
AF = mybir.ActivationFunctionType
AX = mybir.AxisListType.X
OP = mybir.AluOpType

B_FULL, T, V, D = 128, 512, 8000, 256
NB = 16          # sequences per core
NCORES = 8


# ---------------------------------------------------------------------------
# Harness workarounds: walrus in this environment accepts only ONE sync-wait
# per instruction; split extras onto NoOps (BIR json pass) and chunk the Tile
# exit drain. Also register the NTFF profile hook shim so BASS_TRACE=1 works.
# ---------------------------------------------------------------------------
import json as _json

_SW_CTR = [0]


def _split_sync_waits(bir_json: bytes) -> bytes:
    d = _json.loads(bir_json)
    changed = False
    for fn in d.get("functions", []):
        for blk in fn.get("blocks", []):
            new_insts = []
            for inst in blk.get("instructions", []):
                si = inst.get("sync_info")
                waits = (si or {}).get("on_wait") or []
                if len(waits) > 1:
                    changed = True
                    for w in waits[:-1]:
                        _SW_CTR[0] += 1
                        nop = {
                            "engine": inst["engine"],
                            "ins": [],
                            "outs": [],
                            "name": f"I-swsplit-{_SW_CTR[0]}",
                            "opcode": "NoOp",
                            "sync_info": {"on_update": [], "on_wait": [w]},
                        }
                        if "debug" in inst:
                            nop["debug"] = inst["debug"]
                        new_insts.append(nop)
                    si["on_wait"] = [waits[-1]]
                new_insts.append(inst)
            blk["instructions"] = new_insts
    return _json.dumps(d).encode() if changed else bir_json


def _patched_drain_and_barrier(self, tick_clock, wait_clock):
    drain_inst = self.nc.sync.drain()
    wait_clock.add_sem_waits(
        drain_inst.ins, ScopedClock({None: tick_clock.global_clock})
    )
    si = drain_inst.ins.sync_info
    if si is not None and si.on_wait is not None and len(si.on_wait) > 1:
        waits = list(si.on_wait)
        drain_inst.ins.sync_info = bass_rust.SyncInfo(
            on_wait=waits[:1], on_update=list(si.on_update or [])
        )
        for i in range(1, len(waits)):
            nop = self.nc.sync.nop()
            nop.ins.sync_info = bass_rust.SyncInfo(on_wait=[waits[i]], on_update=[])
    self.nc.all_engine_barrier()
    assert self.sems is not None
    popped = self.nc._tile_sem_poison_stack.pop()
    assert popped is self._sem_poison
    self.nc.clear_and_free_semaphores(list(self.sems.allocated().values()))
    self.nc.all_engine_barrier()


_PATCHED = [False]


def _apply_patches():
    if _PATCHED[0]:
        return
    _PATCHED[0] = True
    tile.TileContext._drain_and_barrier = _patched_drain_and_barrier
    import concourse.bass_utils as _bu
    import concourse.bass2jax as _b2j

    _orig_compile = _bu.compile_bir_kernel

    def _wrapped(bir_json, tmpdir, neff_name="file.neff"):
        return _orig_compile(_split_sync_waits(bir_json), tmpdir, neff_name)

    _wrapped._swsplit_wrapped = True
    _bu.compile_bir_kernel = _wrapped
    _b2j.compile_bir_kernel = _wrapped

    if "antenv.axon_hooks" not in sys.modules:
        try:
            import trn_agent_boot.trn_boot as _tb
            _hook = _tb._ntff_profile_via_ctypes("/opt/axon/libaxon_pjrt.so")
        except Exception:
            _hook = None
        m = types.ModuleType("antenv.axon_hooks")
        m.get_axon_ntff_profile_hook = lambda: _hook
        m.set_axon_ntff_profile_hook = lambda h: None
        sys.modules["antenv.axon_hooks"] = m





def A(t, off, dims, p0=0):
    # t: pool tile AP [[rowsize, P], [1, rowsize]]. dims[0] is the partition
    # pair whose step is replaced by the tile's canonical per-partition row
    # size; off is the within-partition element offset.
    rs = t.ap[0][0]
    d = [list(x) for x in dims]
    d[0] = [rs, d[0][1]]
    return bass.AP(t.tensor, t.offset + p0 * rs + off, d)


def AD(handle, off, dims):
    return bass.AP(handle, off, [list(d) for d in dims])


def load_crf_consts(nc, dr, pool):
    """Allocate + DMA the lens-dependent / constant CRF tables early (during
    the recurrence) so the CRF phase never waits on them; vmask alone is
    512KB (~20us on one DMA queue)."""
    c = {}
    def bcast(name, p, w, dt=F32):
        t = pool.tile([p, w], dt, name="c_" + name)
        nc.sync.dma_start(t[:], dr[name][None, :].to_broadcast((p, w)))
        c[name] = t
    bcast("transb16", 128, 16)
    bcast("impflat", 128, 16)
    bcast("fromBp4", 128, 4)
    bcast("toEOS4", 16, 4)
    bcast("c3lab4", 16, 4)
    t = pool.tile([16, 4], F32, name="c_wiota4_16")
    nc.sync.dma_start(t[:], dr["wiota4"][None, :].to_broadcast((16, 4)))
    c["wiota4_16"] = t
    bcast("c3p16", 128, 16)
    bcast("i4flat", 128, 16)
    t2 = pool.tile([128, 4], F32, name="c_wiota4_128")
    nc.sync.dma_start(t2[:], dr["wiota4"][None, :].to_broadcast((128, 4)))
    c["wiota4_128"] = t2
    for name, sh, dt in (("vmask128", [128, 1024], I32),
                         ("meq128", [128, 64], I32),
                         ("mlt128", [128, 64], I32),
                         ("outmask128", [128, 64], F32)):
        t = pool.tile(sh, dt, name="c_" + name)
        nc.sync.dma_start(t[:], dr[name][:])
        c[name] = t
    return c


def emit_crf(nc, tc, dr, pool, cs):
    """dr: dict of DRAM handles. pool: sbuf tile pool to allocate from."""
    v = nc.vector

    # ---- V0: build T matrices ------------------------------------------
    scT = pool.tile([128, 256], F32)   # (chpos, s, c)
    # scores_dram is tok-major [8192, 4]: addr = (t*16+b)*4 + c
    for g in range(8):
        nc.sync.dma_start(
            A(scT, 0, [[1, 16], [4, 64], [1, 4]], p0=g * 16),
            AD(dr["scores"], g * 4096, [[4, 16], [64, 64], [1, 4]]),
        )
    transb_sb = cs["transb16"]
    imp_sb = cs["impflat"]
    vmask_sb = cs["vmask128"]
    fromBp_sb = cs["fromBp4"]

    Traw = pool.tile([128, 1024], F32)  # (chpos, s, p, c)
    v.tensor_add(
        out=A(Traw, 0, [[1, 128], [256, 4], [16, 16], [4, 4], [1, 4]]),
        in0=A(scT, 0, [[1, 128], [64, 4], [4, 16], [0, 4], [1, 4]]),
        in1=A(transb_sb, 0, [[1, 128], [0, 4], [0, 16], [4, 4], [1, 4]]),
    )
    T128 = pool.tile([128, 1024], F32)
    v.select(
        out=A(T128, 0, [[1, 128], [256, 4], [16, 16], [4, 4], [1, 4]]),
        mask=A(vmask_sb, 0, [[1, 128], [256, 4], [16, 16], [4, 4], [1, 4]]),
        on_true=A(Traw, 0, [[1, 128], [256, 4], [16, 16], [4, 4], [1, 4]]),
        on_false=A(imp_sb, 0, [[1, 128], [0, 4], [0, 16], [4, 4], [1, 4]]),
    )
    # step 0 (partitions 0:16, chpos=0, s=0): T = e0 + fromBp (rows equal)
    v.tensor_add(
        out=A(T128, 0, [[1, 16], [4, 4], [1, 4]]),
        in0=A(scT, 0, [[1, 16], [0, 4], [1, 4]]),
        in1=A(fromBp_sb, 0, [[1, 16], [0, 4], [1, 4]]),
    )

    # ---- V1: chunk max-plus products -----------------------------------
    Ma = pool.tile([128, 64], F32)   # (chpos, i, k/j)
    Mb = pool.tile([128, 64], F32)
    tmp256 = pool.tile([128, 1024], F32)
    v.tensor_copy(
        A(Ma, 0, [[1, 128], [16, 4], [4, 4], [1, 4]]),
        A(T128, 0, [[1, 128], [256, 4], [4, 4], [1, 4]]),
    )
    cur, nxt = Ma, Mb
    for s in range(1, 16):
        v.tensor_add(
            out=A(tmp256, 0, [[1, 128], [64, 4], [16, 4], [4, 4], [1, 4]]),
            in0=A(cur, 0, [[1, 128], [16, 4], [4, 4], [1, 4], [0, 4]]),
            in1=A(T128, s * 16, [[1, 128], [256, 4], [0, 4], [4, 4], [1, 4]]),
        )
        v.tensor_reduce(
            out=A(nxt, 0, [[1, 128], [16, 4], [4, 4], [1, 4]]),
            in_=A(tmp256, 0, [[1, 128], [64, 4], [16, 4], [1, 4], [4, 4]]),
            axis=AX, op=OP.max,
        )
        cur, nxt = nxt, cur
    nc.sync.dma_start(
        AD(dr["mdram"], 0, [[64, 128], [1, 64]]),
        A(cur, 0, [[1, 128], [1, 64]]),
    )

    # ---- V2: serial chunk scan (16 partitions) -------------------------
    M16 = pool.tile([16, 512], F32)
    for g in range(8):
        nc.sync.dma_start(
            A(M16, g * 64, [[1, 16], [1, 64]]),
            AD(dr["mdram"], g * 1024, [[64, 16], [1, 64]]),
        )
    Ball = pool.tile([16, 132], F32)
    v.memset(Ball[:], 0.0)
    t16 = pool.tile([16, 16], F32)
    for c in range(32):
        v.tensor_add(
            out=A(t16, 0, [[1, 16], [4, 4], [1, 4]]),
            in0=A(Ball, c * 4, [[1, 16], [1, 4], [0, 4]]),
            in1=A(M16, c * 16, [[1, 16], [4, 4], [1, 4]]),
        )
        v.tensor_reduce(
            out=A(Ball, (c + 1) * 4, [[1, 16], [1, 4]]),
            in_=A(t16, 0, [[1, 16], [1, 4], [4, 4]]),
            axis=AX, op=OP.max,
        )
    # last label one-hot
    toEOS_sb = cs["toEOS4"]
    c3lab_sb = cs["c3lab4"]
    wiota16 = cs["wiota4_16"]
    fin = pool.tile([16, 4], F32)
    v.tensor_add(out=fin[:], in0=A(Ball, 128, [[1, 16], [1, 4]]), in1=toEOS_sb[:])
    lmax = pool.tile([16, 1], F32)
    v.tensor_reduce(out=lmax[:], in_=fin[:], axis=AX, op=OP.max)
    loh = pool.tile([16, 4], F32)
    v.tensor_tensor(out=loh[:], in0=fin[:],
                    in1=A(lmax, 0, [[1, 16], [0, 4]]), op=OP.is_equal)
    lohm = pool.tile([16, 4], F32)
    v.tensor_mul(out=lohm[:], in0=loh[:], in1=c3lab_sb[:])
    lenc = pool.tile([16, 1], F32)
    v.tensor_reduce(out=lenc[:], in_=lohm[:], axis=AX, op=OP.max)
    llval = pool.tile([16, 1], F32)
    v.tensor_scalar(out=llval[:], in0=lenc[:], scalar1=-1.0, scalar2=3.0,
                    op0=OP.mult, op1=OP.add)
    lloh = pool.tile([16, 4], F32)
    v.tensor_tensor(out=lloh[:], in0=wiota16[:],
                    in1=A(llval, 0, [[1, 16], [0, 4]]), op=OP.is_equal)
    nc.sync.dma_start(AD(dr["lldram"], 0, [[4, 16], [1, 4]]), lloh[:])
    nc.sync.dma_start(AD(dr["edram"], 0, [[132, 16], [1, 132]]), Ball[:])

    # ---- V3: replay -> backtrace tables --------------------------------
    c3p_sb = cs["c3p16"]
    bestA = pool.tile([128, 16], F32)
    bestB = pool.tile([128, 16], F32)
    nc.sync.dma_start(
        bestA[:], AD(dr["edram"], 0, [[16, 8], [132, 16], [4, 4], [1, 4]])
    )
    BT = pool.tile([128, 256], F32)     # (chpos, s, c)
    smat = pool.tile([128, 64], F32)
    oh64 = pool.tile([128, 64], F32)
    enc128 = pool.tile([128, 16], F32)
    bcur, bnxt = bestA, bestB
    for s in range(16):
        v.tensor_add(
            out=A(smat, 0, [[1, 128], [16, 4], [4, 4], [1, 4]]),
            in0=A(bcur, 0, [[1, 128], [4, 4], [1, 4], [0, 4]]),
            in1=A(T128, s * 16, [[1, 128], [256, 4], [4, 4], [1, 4]]),
        )
        v.tensor_reduce(
            out=A(bnxt, 0, [[1, 128], [4, 4], [1, 4]]),
            in_=A(smat, 0, [[1, 128], [16, 4], [1, 4], [4, 4]]),
            axis=AX, op=OP.max,
        )
        v.tensor_tensor(
            out=A(oh64, 0, [[1, 128], [16, 4], [4, 4], [1, 4]]),
            in0=A(smat, 0, [[1, 128], [16, 4], [4, 4], [1, 4]]),
            in1=A(bnxt, 0, [[1, 128], [4, 4], [0, 4], [1, 4]]),
            op=OP.is_equal,
        )
        v.tensor_mul(
            out=A(oh64, 0, [[1, 128], [16, 4], [4, 4], [1, 4]]),
            in0=A(oh64, 0, [[1, 128], [16, 4], [4, 4], [1, 4]]),
            in1=A(c3p_sb, 0, [[1, 128], [0, 4], [4, 4], [1, 4]]),
        )
        v.tensor_reduce(
            out=A(enc128, 0, [[1, 128], [4, 4], [1, 4]]),
            in_=A(oh64, 0, [[1, 128], [16, 4], [1, 4], [4, 4]]),
            axis=AX, op=OP.max,
        )
        v.tensor_scalar(
            out=A(BT, s * 4, [[1, 128], [64, 4], [1, 4]]),
            in0=A(enc128, 0, [[1, 128], [4, 4], [1, 4]]),
            scalar1=-1.0, scalar2=3.0, op0=OP.mult, op1=OP.add,
        )
        bcur, bnxt = bnxt, bcur
    nc.sync.dma_start(
        AD(dr["btdram"], 0, [[256, 128], [1, 256]]),
        A(BT, 0, [[1, 128], [1, 256]]),
    )

    # ---- VA: backtrace map tables + chunk compositions -----------------
    BTS = pool.tile([128, 256], F32)
    # top group's last slot is never used; zero-fill before partial overwrite
    v.memset(A(BTS, 252, [[1, 128], [1, 4]]), 0.0)
    nc.sync.dma_start(
        A(BTS, 0, [[1, 128], [1, 252]]),
        AD(dr["btdram"], 4, [[256, 128], [1, 252]]),
    )
    # last slot of each partition: first bt entry of the next chunk group
    nc.sync.dma_start(
        A(BTS, 252, [[1, 112], [1, 4]]),
        AD(dr["btdram"], 16 * 256, [[256, 112], [1, 4]]),
    )
    meq_sb = cs["meq128"]
    mlt_sb = cs["mlt128"]
    lloh128 = pool.tile([128, 4], F32)
    nc.sync.dma_start(lloh128[:], AD(dr["lldram"], 0, [[0, 8], [4, 16], [1, 4]]))
    i4_sb = cs["i4flat"]
    wiota128 = cs["wiota4_128"]

    Fall = pool.tile([128, 1024], F32)  # (chpos, s, u, w)
    tmpA = pool.tile([128, 64], F32)
    for s in range(16):
        # oh(u,w) = bt_{t+1}[u] == w
        v.tensor_tensor(
            out=A(tmpA, 0, [[1, 128], [16, 4], [4, 4], [1, 4]]),
            in0=A(BTS, s * 4, [[1, 128], [64, 4], [1, 4], [0, 4]]),
            in1=A(wiota128, 0, [[1, 128], [0, 4], [0, 4], [1, 4]]),
            op=OP.is_equal,
        )
        # tmp2 = meq ? lloh : I4  ; F = mlt ? oh : tmp2  (write into Fall)
        v.select(
            out=A(Fall, s * 16, [[1, 128], [256, 4], [4, 4], [1, 4]]),
            mask=A(meq_sb, s, [[1, 128], [16, 4], [0, 4], [0, 4]]),
            on_true=A(lloh128, 0, [[1, 128], [0, 4], [0, 4], [1, 4]]),
            on_false=A(i4_sb, 0, [[1, 128], [0, 4], [4, 4], [1, 4]]),
        )
        v.select(
            out=A(Fall, s * 16, [[1, 128], [256, 4], [4, 4], [1, 4]]),
            mask=A(mlt_sb, s, [[1, 128], [16, 4], [0, 4], [0, 4]]),
            on_true=A(tmpA, 0, [[1, 128], [16, 4], [4, 4], [1, 4]]),
            on_false=A(Fall, s * 16, [[1, 128], [256, 4], [4, 4], [1, 4]]),
        )
    Ga = pool.tile([128, 64], F32)
    Gb = pool.tile([128, 64], F32)
    v.tensor_copy(
        A(Ga, 0, [[1, 128], [16, 4], [4, 4], [1, 4]]),
        A(Fall, 15 * 16, [[1, 128], [256, 4], [4, 4], [1, 4]]),
    )
    gcur, gnxt = Ga, Gb
    for s in range(14, -1, -1):
        v.tensor_mul(
            out=A(tmp256, 0, [[1, 128], [64, 4], [16, 4], [4, 4], [1, 4]]),
            in0=A(gcur, 0, [[1, 128], [16, 4], [4, 4], [1, 4], [0, 4]]),
            in1=A(Fall, s * 16, [[1, 128], [256, 4], [0, 4], [4, 4], [1, 4]]),
        )
        v.tensor_reduce(
            out=A(gnxt, 0, [[1, 128], [16, 4], [4, 4], [1, 4]]),
            in_=A(tmp256, 0, [[1, 128], [64, 4], [16, 4], [1, 4], [4, 4]]),
            axis=AX, op=OP.max,
        )
        gcur, gnxt = gnxt, gcur
    nc.sync.dma_start(
        AD(dr["gdram"], 0, [[64, 128], [1, 64]]),
        A(gcur, 0, [[1, 128], [1, 64]]),
    )

    # ---- VB: serial reverse chunk scan (16 partitions) -----------------
    Gall16 = pool.tile([16, 512], F32)
    for g in range(8):
        nc.sync.dma_start(
            A(Gall16, g * 64, [[1, 16], [1, 64]]),
            AD(dr["gdram"], g * 1024, [[64, 16], [1, 64]]),
        )
    EB = pool.tile([16, 132], F32)
    nc.sync.dma_start(
        A(EB, 128, [[1, 16], [1, 4]]), dr["e0oh4"][None, :].to_broadcast((16, 4))
    )
    tb16 = pool.tile([16, 16], F32)
    for c in range(31, -1, -1):
        v.tensor_mul(
            out=tb16[:],
            in0=A(EB, (c + 1) * 4, [[1, 16], [1, 4], [0, 4]]),
            in1=A(Gall16, c * 16, [[1, 16], [4, 4], [1, 4]]),
        )
        v.tensor_reduce(
            out=A(EB, c * 4, [[1, 16], [1, 4]]),
            in_=A(tb16, 0, [[1, 16], [1, 4], [4, 4]]),
            axis=AX, op=OP.max,
        )
    nc.sync.dma_start(AD(dr["ebdram"], 0, [[132, 16], [1, 132]]), EB[:])

    # ---- VC: labels -----------------------------------------------------
    cohE = pool.tile([128, 16], F32)
    nc.sync.dma_start(
        cohE[:], AD(dr["ebdram"], 4, [[16, 8], [132, 16], [4, 4], [1, 4]])
    )
    LABOH = pool.tile([128, 256], F32)  # (chpos, s, w)
    tmpc = pool.tile([128, 64], F32)
    for s in range(15, -1, -1):
        if s == 15:
            in0 = A(cohE, 0, [[1, 128], [4, 4], [1, 4], [0, 4]])
        else:
            in0 = A(LABOH, (s + 1) * 4, [[1, 128], [64, 4], [1, 4], [0, 4]])
        v.tensor_mul(
            out=A(tmpc, 0, [[1, 128], [16, 4], [4, 4], [1, 4]]),
            in0=in0,
            in1=A(Fall, s * 16, [[1, 128], [256, 4], [4, 4], [1, 4]]),
        )
        v.tensor_reduce(
            out=A(LABOH, s * 4, [[1, 128], [64, 4], [1, 4]]),
            in_=A(tmpc, 0, [[1, 128], [16, 4], [1, 4], [4, 4]]),
            axis=AX, op=OP.max,
        )
    omask_sb = cs["outmask128"]
    labv = pool.tile([128, 64], F32)
    tmpl = pool.tile([128, 256], F32)
    v.tensor_mul(
        out=A(tmpl, 0, [[1, 128], [64, 4], [4, 16], [1, 4]]),
        in0=A(LABOH, 0, [[1, 128], [64, 4], [4, 16], [1, 4]]),
        in1=A(wiota128, 0, [[1, 128], [0, 4], [0, 16], [1, 4]]),
    )
    v.tensor_reduce(
        out=A(labv, 0, [[1, 128], [16, 4], [1, 16]]),
        in_=A(tmpl, 0, [[1, 128], [64, 4], [4, 16], [1, 4]]),
        axis=AX, op=OP.add,
    )
    v.tensor_mul(out=labv[:], in0=labv[:], in1=omask_sb[:])
    labi = pool.tile([128, 64], I32)
    v.tensor_copy(labi[:], labv[:])
    for cp in range(4):
        nc.sync.dma_start(
            AD(dr["labels"], 16 * cp, [[64, 8], [512, 16], [1, 16]]),
            A(labi, cp * 16, [[1, 128], [1, 16]]),
        )


def host_crf_consts(lens, trans, fromB, toEOS, b_lab):
    """All host-side constant arrays, keyed to match dram handle names."""
    import numpy as np
    T, B, L = 512, 16, 4
    NEG = -1e9
    chgrp = np.arange(8)
    out = {}
    out["transb16"] = (trans + b_lab[None, :]).astype(np.float32).reshape(16)
    imp = np.full((L, L), NEG, np.float32)
    np.fill_diagonal(imp, 0.0)
    out["impflat"] = imp.reshape(16)
    out["fromBp4"] = (fromB + b_lab).astype(np.float32)
    out["toEOS4"] = toEOS.astype(np.float32)
    out["c3lab4"] = (3.0 - np.arange(4)).astype(np.float32)
    out["wiota4"] = np.arange(4).astype(np.float32)
    out["c3p16"] = np.repeat(3.0 - np.arange(4), 4).astype(np.float32)
    out["i4flat"] = np.eye(4, dtype=np.float32).reshape(16)
    out["e0oh4"] = np.array([1, 0, 0, 0], np.float32)
    # t value at (P, chpos, s):  P = chgrp*16 + b ; t = 16*(4*chgrp+chpos)+s
    P_chgrp = np.arange(128) // 16
    P_b = np.arange(128) % 16
    chpos = np.arange(4)
    s = np.arange(16)
    tt = 16 * (4 * P_chgrp[:, None, None] + chpos[None, :, None]) + s[None, None, :]
    lb = lens[P_b][:, None, None]
    vm = (tt < lb)
    out["vmask128"] = np.repeat(
        vm.reshape(128, 64)[:, :, None], 16, axis=2
    ).reshape(128, 1024).astype(np.int32)
    out["meq128"] = (tt == lb - 1).reshape(128, 64).astype(np.int32)
    out["mlt128"] = (tt < lb - 1).reshape(128, 64).astype(np.int32)
    out["outmask128"] = (tt < lb).reshape(128, 64).astype(np.float32)
    return out


CRF_DRAM_SPECS = [
    ("transb16", [16], F32), ("impflat", [16], F32), ("fromBp4", [4], F32),
    ("toEOS4", [4], F32), ("c3lab4", [4], F32), ("wiota4", [4], F32),
    ("c3p16", [16], F32), ("i4flat", [16], F32), ("e0oh4", [4], F32),
    ("vmask128", [128, 1024], I32), ("meq128", [128, 64], I32),
    ("mlt128", [128, 64], I32), ("outmask128", [128, 64], F32),
]
CRF_SCRATCH_SPECS = [
    ("mdram", [8192], F32), ("edram", [2112], F32), ("btdram", [32832], F32),
    ("gdram", [8192], F32), ("lldram", [64], F32), ("ebdram", [2112], F32),
]






class LstmEmitter:
    def __init__(self, nc, tc, dr, T, pools):
        self.nc, self.tc, self.dr, self.T = nc, tc, dr, T
        self.NBLK = T // 64
        p = pools
        hist_all = p["hist"].tile([128, 2 * (T + 1) * 16], F16, name="hist_all")
        self.hist_all = hist_all
        self.hb_off = (T + 1) * 16
        self.hist = {
            "f": hist_all[:, 0:(T + 1) * 16],
            "b": hist_all[:, (T + 1) * 16:2 * (T + 1) * 16],
        }
        nc.vector.memset(self.hist["f"][:, 0:16], 0.0)
        nc.vector.memset(self.hist["b"][:, T * 16:(T + 1) * 16], 0.0)
        # tall layout (dir-major, doubled cell state C=2c interleaved so
        # every DVE operand is a <=3-dim AP):
        #   [i_f f_f o_f g_f](0:64) [C_f](64:80) [i_b f_b o_b g_b](80:144)
        #   [C_b](144:160)
        self.tall = p["state"].tile([128, 160], F16, name="tall")
        nc.vector.memset(A(self.tall, 64, [[1, 128], [80, 2], [1, 16]]), 0.0)
        self.g_ps_by_slot = {}
        # weights
        self.whhT = {}
        self.wihT = {}
        self.biasT = {}
        self.wlabT = {}
        self.pools = p
        self.wx = {}   # (dir, blk) -> tile [128, 4096] fp16, chunk-major

    # Weight/const DMAs are emitted AFTER the prologue's gather DMAs so the
    # tiny token-index DMAs are not stuck behind ~40us of weight traffic.
    def load_ident(self):
        nc, dr, p = self.nc, self.dr, self.pools
        self.ident = p["wts"].tile([128, 128], F16, name="ident")
        nc.scalar.dma_start(self.ident[:], dr["ident"][:])

    def load_weights(self):
        nc, dr, p = self.nc, self.dr, self.pools
        for d in ("f", "b"):
            w = p["wts"].tile([128, 512], F16, name=f"whh_{d}")
            nc.sync.dma_start(w[:], dr[f"whhT_{d}"][:])
            self.whhT[d] = w
            hs = []
            for h in range(2):
                wh = p["wts"].tile([128, 512], F16, name=f"wih_{d}{h}")
                nc.sync.dma_start(wh[:], dr[f"wihT_{d}{h}"][:])
                hs.append(wh)
            self.wihT[d] = hs
            bt = p["wts"].tile([128, 4], F32, name=f"bias_{d}")
            nc.sync.dma_start(bt[:], dr[f"biasT_{d}"][:])
            self.biasT[d] = bt
            wl = p["wts"].tile([128, 4], F16, name=f"wlab_{d}")
            nc.sync.dma_start(wl[:], dr[f"wlabT_{d}"][:])
            self.wlabT[d] = wl
        self.masktab = p["wts"].tile([1, 512 * 64], F16, name="masktab")
        nc.sync.dma_start(self.masktab[:], dr["masktab"][None, :])
        self.ones1 = p["wts"].tile([1, 128], F16, name="ones1")
        nc.vector.memset(self.ones1[:], 1.0)

    # ---- production of one dir-block's wx ------------------------------
    def production_items(self, d, blk):
        """Returns a list of closures emitting the gather/transpose/matmul/
        bias work that materializes wx[d][blk]."""
        nc, dr, p = self.nc, self.dr, self.pools
        items = []
        xg = []      # gather tiles [128, 256] fp16
        xt = None    # XT tile [128, 2048] fp16 (halves at 0 / 1024)
        wxt = None   # wx out tile [128, 4096] fp16
        state = {}

        def alloc():
            state["xt"] = p["xt"].tile([128, 2048], F16, name="xt")
            state["wx"] = p[f"wx_{d}"].tile([128, 4096], F16, name=f"wx_{d}")
            self.wx[(d, blk)] = state["wx"]

        idxs = {}

        def gather_idx(i):
            def go():
                idx = p["idx"].tile([128, 1], I32)
                nc.sync.dma_start(
                    idx[:],
                    bass.AP(dr["tokens"], blk * 1024 + i * 128, [[1, 128], [1, 1]]),
                )
                idxs[i] = idx
            return go

        def gather(i):
            def go():
                t = p["xg"].tile([128, 256], F16)
                nc.gpsimd.indirect_dma_start(
                    out=t[:], out_offset=None, in_=dr["emb16"][:],
                    in_offset=bass.IndirectOffsetOnAxis(ap=idxs.pop(i)[:, :1], axis=0),
                )
                xg.append(t)
            return go

        def transp(i, h):
            def go():
                ps = p["tp_ps"].tile([128, 128], F16)
                nc.tensor.transpose(
                    out=ps[:], in_=xg[i][:, h * 128:(h + 1) * 128],
                    identity=self.ident[:],
                )
                # psum->sbuf move on DVE (gpsimd has no PSUM port) to keep
                # the scalar engine free for the recurrence chain
                nc.vector.tensor_copy(
                    state["xt"][:, h * 1024 + i * 128: h * 1024 + (i + 1) * 128],
                    ps[:],
                )
            return go

        def mm(j, n):
            def go():
                ps = p["wx_ps"].tile([128, 512], F32)
                state[("ps", j, n)] = ps
                for h in range(2):
                    nc.tensor.matmul(
                        out=ps[:],
                        lhsT=self.wihT[d][h][:, j * 128:(j + 1) * 128],
                        rhs=state["xt"][:, h * 1024 + n * 512: h * 1024 + (n + 1) * 512],
                        start=(h == 0), stop=(h == 1),
                    )
            return go

        def bias(j, n):
            def go():
                nc.scalar.activation(
                    out=state["wx"][:, j * 1024 + n * 512: j * 1024 + (n + 1) * 512],
                    in_=state[("ps", j, n)][:],
                    func=AF.Identity, bias=self.biasT[d][:, j:j + 1],
                )
            return go

        items.append(alloc)
        for i in range(8):
            items.append(gather_idx(i))
        for i in range(8):
            items.append(gather(i))
        for i in range(8):
            for h in range(2):
                items.append(transp(i, h))
        for j in range(4):
            for n in range(2):
                items.append(mm(j, n))
                items.append(bias(j, n))
        return items

    # ---- one recurrence slot: fwd step t_f and bwd step t_b merged ----
    # All-tanh cell: host prep halves i/f/o pre-activations, so
    # sigma(x) = (tanh(x/2)+1)/2 and one Tanh covers all 4 gates. Hidden
    # state is tracked doubled (H = 2h, Whh/W_lab pre-halved) and cell
    # state doubled (C = 2c).
    # PSUM layout dir-major: cols 0:64 fwd gates [i f o g]x16, 64:128 bwd.
    # The wx prefill matmuls have no H dependence, so the in-order PE
    # executes them during the previous slot's ACT/DVE phase.
    def slot(self, t_f):
        nc, p, T = self.nc, self.pools, self.T
        t_b = T - 1 - t_f
        ha = self.hist_all
        hb0 = self.hb_off
        hprev = {"f": ha[:, t_f * 16:(t_f + 1) * 16],
                 "b": ha[:, hb0 + (t_b + 1) * 16:hb0 + (t_b + 2) * 16]}
        g_ps = self.g_ps_by_slot.pop(t_f)
        # The wx prefill was written into the psum bank by ACT/DVE one slot
        # ago, so the whole matmul group is start=False on top of it (a
        # matmul prefill is not hoistable: a second start=True drops the
        # first group's prefill, verified on hardware).
        # bwd pack-padding: add -15 to i/f/o pre-activations at t >= len
        # via a K=1 matmul from a tiny per-t table. No H dependence, so
        # the in-order PE runs it before the H-dependent whh run.
        nc.tensor.matmul(
            out=g_ps[:, 64:128],
            lhsT=self.ones1[:],
            rhs=A(self.masktab, t_b * 64, [[1, 1], [1, 64]]),
            start=False, stop=False,
        )
        for d, off in (("f", 0), ("b", 64)):
            for j in range(4):
                nc.tensor.matmul(
                    out=g_ps[:, off + j * 16:off + (j + 1) * 16],
                    lhsT=self.whhT[d][:, j * 128:(j + 1) * 128],
                    rhs=hprev[d], start=False,
                    stop=(d == "b" and j == 3),
                )
        tall = self.tall
        nc.scalar.activation(
            out=A(tall, 0, [[1, 128], [80, 2], [1, 64]]),
            in_=g_ps[:], func=AF.Tanh,
        )
        # m12 = (in0+1)*in1 with in0 = [ti|tf], in1 = [tg|C] per dir
        # -> m12 cols: [m1_f](0:16) [m2_f](16:32) [m1_b](32:48) [m2_b](48:64)
        m12 = p["m12"].tile([128, 64], F16, name="m12")
        nc.vector.scalar_tensor_tensor(
            out=m12[:],
            in0=A(tall, 0, [[1, 128], [80, 2], [1, 32]]),
            scalar=1.0,
            in1=A(tall, 48, [[1, 128], [80, 2], [1, 32]]),
            op0=OP.add, op1=OP.mult,
        )
        # C_new = 2c_new = (ti+1)*tg + (tf+1)*C/2
        nc.vector.scalar_tensor_tensor(
            out=A(tall, 64, [[1, 128], [80, 2], [1, 16]]),
            in0=A(m12, 16, [[1, 128], [32, 2], [1, 16]]),
            scalar=0.5,
            in1=A(m12, 0, [[1, 128], [32, 2], [1, 16]]),
            op0=OP.mult, op1=OP.add,
        )
        tc_ = p["tc"].tile([128, 32], F16, name="tc")
        nc.scalar.activation(
            out=tc_[:], in_=A(tall, 64, [[1, 128], [80, 2], [1, 16]]),
            func=AF.Tanh, scale=0.5,
        )
        # H = (to+1)*tanh(c) = 2h, stored to both dirs' history in one op
        dlt = hb0 + t_b * 16 - (t_f + 1) * 16
        nc.vector.scalar_tensor_tensor(
            out=A(ha, (t_f + 1) * 16, [[1, 128], [dlt, 2], [1, 16]]),
            in0=A(tall, 32, [[1, 128], [80, 2], [1, 16]]),
            scalar=1.0,
            in1=tc_[:],
            op0=OP.add, op1=OP.mult,
        )
        if t_f + 1 < T:
            self.prefill(t_f + 1)

    # wx psum prefill for slot t: fwd half written by the scalar engine,
    # bwd half by DVE. Both run in their engines' idle window (no H
    # dependence) and complete before the PE's start=False matmul group
    # accumulates on top.
    def prefill(self, t_f):
        nc, p, T = self.nc, self.pools, self.T
        t_b = T - 1 - t_f
        g_ps = p["gps"].tile([128, 128], F32, name="g_ps")
        self.g_ps_by_slot[t_f] = g_ps
        wxf = self.wx[("f", t_f // 64)]
        nc.scalar.activation(
            out=g_ps[:, 0:64],
            in_=A(wxf, (t_f % 64) * 16, [[1, 128], [1024, 4], [1, 16]]),
            func=AF.Copy,
        )
        wxb = self.wx[("b", t_b // 64)]
        nc.vector.tensor_copy(
            g_ps[:, 64:128],
            A(wxb, (t_b % 64) * 16, [[1, 128], [1024, 4], [1, 16]]),
        )

    # ---- full pipelined emission ---------------------------------------
    def emit_recurrence(self):
        T, NBLK = self.T, self.NBLK
        # prologue: produce fwd block 0 and bwd block NBLK-1, interleaved so
        # the gathers/transposes/matmuls of both blocks overlap
        pf = self.production_items("f", 0)
        pb = self.production_items("b", NBLK - 1)
        # items: [0]=alloc, [1:9]=gathers, [9:]=transposes/matmuls/bias.
        # All 16 gather DMAs go first (the serial gpsimd queue is the
        # prologue's critical path), then the weight loads, then the rest.
        self.load_ident()
        pf[0](); pb[0]()
        for i in range(1, 9):
            pf[i]()
        for i in range(1, 9):
            pb[i]()
        self.load_weights()
        self.crf_consts = load_crf_consts(self.nc, self.dr,
                                          self.pools["wts"])
        for i in range(9, max(len(pf), len(pb))):
            if i < len(pf):
                pf[i]()
            if i < len(pb):
                pb[i]()
        self.prefill(0)
        # score chunk n becomes computable after slot max(8n+7, T-1-8n)
        ready = {}
        for n in range(T * 16 // 128):
            ready.setdefault(max(8 * n + 7, T - 1 - 8 * n), []).append(n)
        for blk in range(NBLK):
            todo = []
            if blk + 1 < NBLK:
                todo += self.production_items("f", blk + 1)
                todo += self.production_items("b", NBLK - 2 - blk)
            k = 0
            for tin in range(64):
                t = blk * 64 + tin
                if tin == 63:
                    # the next block's first slot reads the next blocks' wx:
                    # emit every production item first
                    while k < len(todo):
                        todo[k]()
                        k += 1
                self.slot(t)
                for n in ready.get(t, []):
                    self.emit_score_chunk(n)
                want = ((tin + 1) * len(todo)) // 64
                while k < want:
                    todo[k]()
                    k += 1

    def emit_score_chunk(self, n):
        # emission scores for t in [8n, 8n+8): ready once hf[8n+7] (slot
        # 8n+7) and hb[8n] (slot T-1-8n) both exist
        nc, p = self.nc, self.pools
        ps = p["sc_ps"].tile([128, 4], F32, name="sc_ps")
        nc.tensor.matmul(out=ps[:], lhsT=self.hist["f"][:, 16 + n * 128: 16 + (n + 1) * 128],
                         rhs=self.wlabT["f"][:], start=True, stop=False)
        nc.tensor.matmul(out=ps[:], lhsT=self.hist["b"][:, n * 128:(n + 1) * 128],
                         rhs=self.wlabT["b"][:], start=False, stop=True)
        sb = p["sc_sb"].tile([128, 4], F32, name="sc_sb")
        nc.vector.tensor_copy(sb[:], ps[:])
        nc.sync.dma_start(
            bass.AP(self.dr["scores"], n * 512, [[4, 128], [1, 4]]), sb[:]
        )


def build_masktab(lens, T):
    import numpy as np
    # per-(t, j, seq) additive bwd mask: -15 on i/f/o gates at t >= len[b]
    # (tanh saturates to -1 so the gates close and state self-freezes to 0)
    t = np.arange(T)
    pad = (t[:, None] >= lens[None, :])            # [T, 16]
    m = np.zeros((T, 4, 16), np.float16)
    m[:, :3, :] = np.where(pad, -15.0, 0.0)[:, None, :]
    return m.reshape(-1)


def lstm_dram_specs(T=512):
    return [
        ("emb16", [8000, 256], F16), ("tokens", [T * 16], I32),
        ("wihT_f0", [128, 512], F16), ("wihT_f1", [128, 512], F16),
        ("wihT_b0", [128, 512], F16), ("wihT_b1", [128, 512], F16),
        ("whhT_f", [128, 512], F16), ("whhT_b", [128, 512], F16),
        ("biasT_f", [128, 4], F32), ("biasT_b", [128, 4], F32),
        ("wlabT_f", [128, 4], F16), ("wlabT_b", [128, 4], F16),
        ("ident", [128, 128], F16),
        ("masktab", [T * 64], F16),
    ]


def make_pools(ctx_persist, ctx_trans, tc):
    p = {}
    p["hist"] = ctx_persist.enter_context(tc.tile_pool(name="hist", bufs=1))
    p["state"] = ctx_persist.enter_context(tc.tile_pool(name="state", bufs=1))
    p["wts"] = ctx_persist.enter_context(tc.tile_pool(name="wts", bufs=1))
    p["idx"] = ctx_trans.enter_context(tc.tile_pool(name="idx", bufs=32))
    p["xg"] = ctx_trans.enter_context(tc.tile_pool(name="xg", bufs=16))
    p["xt"] = ctx_trans.enter_context(tc.tile_pool(name="xt", bufs=2))
    p["wx_f"] = ctx_trans.enter_context(tc.tile_pool(name="wx_f", bufs=2))
    p["wx_b"] = ctx_trans.enter_context(tc.tile_pool(name="wx_b", bufs=2))
    p["tp_ps"] = ctx_trans.enter_context(tc.tile_pool(name="tp_ps", bufs=2, space="PSUM"))
    p["wx_ps"] = ctx_trans.enter_context(tc.tile_pool(name="wx_ps", bufs=2, space="PSUM"))
    p["gps"] = ctx_trans.enter_context(tc.tile_pool(name="gps", bufs=3, space="PSUM"))
    for nm in ("m12", "tc"):
        p[nm] = ctx_trans.enter_context(tc.tile_pool(name=nm, bufs=2))
    return p


def make_score_pools(ctx, tc):
    p = {}
    p["sc_ps"] = ctx.enter_context(tc.tile_pool(name="sc_ps", bufs=1, space="PSUM"))
    p["sc_sb"] = ctx.enter_context(tc.tile_pool(name="sc_sb", bufs=2))
    return p


# ---------------------------------------------------------------------------
# DRAM declarations + host prep + SPMD driver
# ---------------------------------------------------------------------------

def _build_program():
    nc = bass.Bass(trn_type="TRN2")
    dr = {}
    for name, shape, dt in lstm_dram_specs(T):
        dr[name] = nc.dram_tensor(name, shape, dt, kind="ExternalInput")
    for name, shape, dt in CRF_DRAM_SPECS:
        dr[name] = nc.dram_tensor(name, shape, dt, kind="ExternalInput")
    for name, shape, dt in CRF_SCRATCH_SPECS:
        dr[name] = nc.dram_tensor(name, shape, dt)
    import os as _os
    if _os.environ.get("BASSDBG_SCORES"):
        dr["scores"] = nc.dram_tensor("scores", [T * 16, 4], F32,
                                      kind="ExternalOutput")
    else:
        dr["scores"] = nc.dram_tensor("scores", [T * 16, 4], F32)
    dr["labels"] = nc.dram_tensor("labels", [NB, T], I32, kind="ExternalOutput")

    with tile.TileContext(nc) as tc:
        with ExitStack() as ctx:
            with ExitStack() as ctx_trans:
                pools = make_pools(ctx, ctx_trans, tc)
                pools.update(make_score_pools(ctx_trans, tc))
                em = LstmEmitter(nc, tc, dr, T, pools)
                em.emit_recurrence()
            with ExitStack() as ctx_crf:
                crf_pool = ctx_crf.enter_context(tc.tile_pool(name="crf", bufs=1))
                emit_crf(nc, tc, dr, crf_pool, em.crf_consts)
    return nc


_CACHE = {}
LAST_EXEC_NS = None


def kernel(**inputs):
    global LAST_EXEC_NS
    _apply_patches()
    from concourse.bass_utils import run_bass_kernel_spmd

    inp = {k: np.asarray(v) for k, v in inputs.items()}
    if "nc" not in _CACHE:
        _CACHE["nc"] = _build_program()
    nc = _CACHE["nc"]

    # shared (batch-independent) host arrays.
    # All-tanh trick: i/f/o rows are pre-halved so sigma(x)=(tanh(x/2)+1)/2
    # comes out of a single Tanh; the hidden state is tracked doubled
    # (H=2h), compensated by halving everything that consumes h.
    shared = {}
    shared["emb16"] = inp["emb"].astype(np.float16)
    perm = np.concatenate([np.arange(128), 128 + np.arange(128),
                           384 + np.arange(128), 256 + np.arange(128)])
    halfrow = np.ones((512, 1), np.float32)
    halfrow[:384] = 0.5          # i,f,o rows (post-perm order i,f,o,g)
    for d, sfx in (("f", "_f"), ("b", "_b")):
        wih = inp[f"W_ih{sfx}"][perm].astype(np.float32) * halfrow
        whh = inp[f"W_hh{sfx}"][perm].astype(np.float32) * halfrow * 0.5
        bias = ((inp[f"b_ih{sfx}"] + inp[f"b_hh{sfx}"])[perm]
                .astype(np.float32) * halfrow[:, 0])
        shared[f"wihT_{d}0"] = np.ascontiguousarray(wih.T[:128]).astype(np.float16)
        shared[f"wihT_{d}1"] = np.ascontiguousarray(wih.T[128:]).astype(np.float16)
        shared[f"whhT_{d}"] = np.ascontiguousarray(whh.T).astype(np.float16)
        shared[f"biasT_{d}"] = np.ascontiguousarray(
            bias.reshape(4, 128).T).astype(np.float32)
        wl = inp["W_lab"].astype(np.float32) * 0.5
        half = wl[:, :128] if d == "f" else wl[:, 128:]
        shared[f"wlabT_{d}"] = np.ascontiguousarray(half.T).astype(np.float16)
    shared["ident"] = np.eye(128, dtype=np.float16)

    trans = inp["transitions"].astype(np.float32)
    fromB = inp["from_BOS"].astype(np.float32)
    toEOS = inp["to_EOS"].astype(np.float32)
    b_lab = inp["b_lab"].astype(np.float32)

    pad_seq = inp["pad_seq"].astype(np.int64)
    lens_full = inp["lens"].astype(np.int64)

    in_maps = []
    for core in range(NCORES):
        b0 = core * NB
        seq = pad_seq[b0:b0 + NB]
        lens = lens_full[b0:b0 + NB]
        m = dict(shared)
        m["tokens"] = np.ascontiguousarray(seq.T).reshape(-1).astype(np.int32)
        m["masktab"] = build_masktab(lens, T)
        m.update(host_crf_consts(lens, trans, fromB, toEOS, b_lab))
        in_maps.append(m)

    res = run_bass_kernel_spmd(nc, in_maps, list(range(NCORES)))
    LAST_EXEC_NS = res.exec_time_ns
    out = np.concatenate([res.results[c]["labels"] for c in range(NCORES)], axis=0)
    return out.astype(np.int32)

